# revision 6
# baseline (speedup 1.0000x reference)
"""Trainium2 Bass kernel for nn_DetectorHelper — transposed-gates design.

Layout: gates computed TRANSPOSED ([gate_dim, batch]) so the weight matrices
are the PE stationary operand (loaded via Fast-Weight-Load at 2-4 elem/cycle/
partition) and every ACT/DVE op runs on all 128 partitions with tiny free
dims, instead of the baseline's [16, 1024] ops on 16 partitions.

Per core (B=16 batch rows):
  - state: hT ring [128, 4x32] bf16 (col = ktile*16+b), cT ring [128, 2x32] f32
  - per step: 1 identity-inject MM adds the precomputed x-part (xg) from SBUF
    into PSUM, then 16 (LDWEIGHTS+MM N=16) pairs accumulate W_hh^T h;
    ACT: sig[128,96]+tanh[128,32] (gate order i,f,o,g via host-side row
    permutation); DVE: 4 ops on [128,32].
  - xg = W_ih_aug @ x_aug precomputed per 32-step block into an SBUF
    double-buffer by dense N=512 MMs interleaved with the previous block.
  - decoder h states stream to DRAM via DMA; output projection is one dense
    hoisted matmul phase at the end.
Host packs weights bf16 (optionally W_hh in fp8-e3m4 x64), x as [65, T*16]
bf16 (aug row of ones), and un-permutes/reverses the [64, T*16] output.
"""

import sys

sys.path.insert(0, "/opt/trn_rl_repo")

from contextlib import ExitStack, nullcontext

import numpy as np

B = 16       # batch rows per core
F = 64       # feature dim
H = 256      # hidden dim
G = 1024     # gate dim
T = 1024
U = 32       # steps per xg block
NBODY = T // (2 * U)   # For_i trip count (2 blocks per body)
N_CORES = 8
WHH_FP8 = True        # W_hh stationary in fp8-e3m4 (x64 scale) instead of bf16
WSCALE = 64.0 if WHH_FP8 else 1.0

_CACHE = {}


def _build(repeat=1):
    import concourse.bass as bass
    import concourse.tile as tile
    from concourse import bacc, mybir

    F32 = mybir.dt.float32
    BF16 = mybir.dt.bfloat16
    WDT = mybir.dt.float8e3 if WHH_FP8 else BF16
    SIG = mybir.ActivationFunctionType.Sigmoid
    TANH = mybir.ActivationFunctionType.Tanh
    COPY = mybir.ActivationFunctionType.Copy
    XCOLS = (T + U) * B    # x padded by one block

    nc = bacc.Bacc("TRN2", target_bir_lowering=False, debug=False,
                   num_devices=N_CORES)

    xte_d = nc.dram_tensor("xte", [F + 1, XCOLS], BF16, kind="ExternalInput").ap()
    xtd_d = nc.dram_tensor("xtd", [F + 1, XCOLS], BF16, kind="ExternalInput").ap()
    wih_e_d = nc.dram_tensor("wih_e", [F + 1, G], BF16, kind="ExternalInput").ap()
    whh_e_d = nc.dram_tensor("whh_e", [128, 16 * 128], WDT, kind="ExternalInput").ap()
    wih_d_d = nc.dram_tensor("wih_d", [F + 1, G], BF16, kind="ExternalInput").ap()
    whh_d_d = nc.dram_tensor("whh_d", [128, 16 * 128], WDT, kind="ExternalInput").ap()
    wout_d = nc.dram_tensor("wout", [128, 128], BF16, kind="ExternalInput").ap()
    bout_d = nc.dram_tensor("bout", [F, 1], F32, kind="ExternalInput").ap()
    ident_d = nc.dram_tensor("ident", [128, 128], BF16, kind="ExternalInput").ap()
    hist_d = nc.dram_tensor("hist", [128, T * 32], BF16, kind="Internal").ap()
    out_d = nc.dram_tensor("out", [F, T * B], F32, kind="ExternalOutput").ap()

    with tile.TileContext(nc) as tc, ExitStack() as ctx:
        wpool = ctx.enter_context(tc.tile_pool(name="wpool", bufs=1))
        wih_e = wpool.tile([F + 1, G], BF16, name="wih_e_sb")
        whh_e = wpool.tile([128, 16 * 128], WDT, name="whh_e_sb")
        wih_d = wpool.tile([F + 1, G], BF16, name="wih_d_sb")
        whh_d = wpool.tile([128, 16 * 128], WDT, name="whh_d_sb")
        wout = wpool.tile([128, 128], BF16, name="wout_sb")
        bout = wpool.tile([F, 1], F32, name="bout_sb")
        ident = wpool.tile([128, 128], BF16, name="ident_sb")
        for sb, dr in [(wih_e, wih_e_d), (whh_e, whh_e_d), (wih_d, wih_d_d),
                       (whh_d, whh_d_d), (wout, wout_d), (bout, bout_d),
                       (ident, ident_d)]:
            nc.sync.dma_start(sb[:], dr[:])

        # persistent state rings + xg double buffer
        hring = wpool.tile([128, 4 * 32], BF16, name="hring")
        cring = wpool.tile([128, 2 * 32], F32, name="cring")
        # [partition, half, chunk, block-col]
        xg_sb = wpool.tile([128, 2, 8, U * B], BF16, name="xg_sb")

        gpool = ctx.enter_context(tc.tile_pool(name="gpool", bufs=2, space="PSUM"))
        xppool = ctx.enter_context(tc.tile_pool(name="xppool", bufs=2, space="PSUM"))
        ppool = ctx.enter_context(tc.tile_pool(name="ppool", bufs=2, space="PSUM"))
        apool = ctx.enter_context(tc.tile_pool(name="apool", bufs=2))
        tpool = ctx.enter_context(tc.tile_pool(name="tpool", bufs=2))
        xspool = ctx.enter_context(tc.tile_pool(name="xspool", bufs=2))
        hspool = ctx.enter_context(tc.tile_pool(name="hspool", bufs=2))
        opool = ctx.enter_context(tc.tile_pool(name="opool", bufs=2))

        def precompute_ops(xt_d, wih, blkexpr, half):
            """Deferred emission list: stage x block, 8 chunk MMs + copies."""
            ops = []
            box = {}

            def dma():
                box["xstg"] = xspool.tile([F + 1, U * B], BF16, name="xstg")
                nc.sync.dma_start(box["xstg"][:], xt_d[:, bass.ts(blkexpr, U * B)])
            ops.append(dma)

            def mk(c):
                def f():
                    xps = xppool.tile([128, U * B], F32, name="xps")
                    nc.tensor.matmul(xps[:], wih[:, 128 * c:128 * c + 128],
                                     box["xstg"][:], start=True, stop=True)
                    dst = xg_sb[:, half, c, :]
                    if c % 2 == 0:
                        nc.vector.tensor_copy(dst, xps[:])
                    else:
                        nc.scalar.activation(dst, xps[:], COPY)
                return f
            ops.extend(mk(c) for c in range(8))
            return ops

        def step(jj, whh, hist_s=None):
            r, w = jj % 4, (jj + 1) % 4
            pc, pw = jj % 2, (jj + 1) % 2
            half, jl = divmod(jj, U)
            hr0 = hring[:, 32 * r:32 * r + 16]
            hr1 = hring[:, 32 * r + 16:32 * r + 32]
            hw = hring[:, 32 * w:32 * w + 32]
            cr = cring[:, 32 * pc:32 * pc + 32]
            cw = cring[:, 32 * pw:32 * pw + 32]

            if hist_s is not None:
                nc.sync.dma_start(hist_d[:, bass.ts(hist_s, 32)],
                                  hring[:, 32 * r:32 * r + 32])

            g_ps = gpool.tile([128, 512], F32, name="g_ps")
            gv = g_ps[:, 0:128]
            # x-part inject: one MM, moving = 8 chunk slivers (strided AP)
            nc.tensor.matmul(gv, ident[:], xg_sb[:, half, :, jl * B:jl * B + B],
                             start=True, stop=False, skip_group_check=True)
            for c in range(8):
                nc.tensor.matmul(g_ps[:, 16 * c:16 * c + 16],
                                 whh[:, (2 * c) * 128:(2 * c) * 128 + 128], hr0,
                                 start=False, stop=False, skip_group_check=True)
                nc.tensor.matmul(g_ps[:, 16 * c:16 * c + 16],
                                 whh[:, (2 * c + 1) * 128:(2 * c + 1) * 128 + 128],
                                 hr1, start=False, stop=True,
                                 skip_group_check=True)

            gact = apool.tile([128, 128], F32, name="gact")
            nc.scalar.activation(gact[:, 0:96], gv[:, 0:96], SIG,
                                 scale=1.0 / WSCALE)
            nc.scalar.activation(gact[:, 96:128], gv[:, 96:128], TANH,
                                 scale=1.0 / WSCALE)
            fc = tpool.tile([128, 32], F32, name="fc")
            nc.vector.tensor_mul(fc[:], gact[:, 32:64], cr)
            ig = tpool.tile([128, 32], F32, name="ig")
            nc.vector.tensor_mul(ig[:], gact[:, 0:32], gact[:, 96:128])
            nc.vector.tensor_add(cw, ig[:], fc[:])
            tch = tpool.tile([128, 32], F32, name="tch")
            nc.scalar.activation(tch[:], cw, TANH)
            nc.vector.tensor_mul(hw, gact[:, 64:96], tch[:])

        def direction(xt_d, wih, whh, i_var, hist_base=None):
            """One 2-block For_i body (64 steps) + interleaved precompute."""
            preq = (precompute_ops(xt_d, wih, 2 * i_var + 1, 1)
                    + precompute_ops(xt_d, wih, 2 * i_var + 2, 0))
            for jj in range(2 * U):
                step(jj, whh,
                     hist_s=None if hist_base is None else hist_base + jj)
                if jj % 3 == 1 and preq:
                    preq.pop(0)()
            while preq:
                preq.pop(0)()

        rep_ctx = tc.For_i(0, repeat) if repeat > 1 else nullcontext()
        with rep_ctx:
            nc.vector.memset(hring[:, 0:32], 0.0)
            nc.vector.memset(cring[:, 0:32], 0.0)

            # ---- encoder ----
            for op in precompute_ops(xte_d, wih_e, 0, 0):
                op()
            with tc.For_i(0, NBODY) as i:
                direction(xte_d, wih_e, whh_e, i)

            # ---- decoder ----
            for op in precompute_ops(xtd_d, wih_d, 0, 0):
                op()
            with tc.For_i(0, NBODY) as i:
                direction(xtd_d, wih_d, whh_d, i, hist_base=i * (2 * U))

            # ---- output projection: out[f, s*16+b] = W_out @ h_s + b_out ----
            with tc.For_i(0, 16) as sl:
                hstg = hspool.tile([128, 2, U, 32], BF16, name="hstg")
                nc.sync.dma_start(hstg[:], hist_d[:, bass.ts(sl, 2 * U * 32)])
                for hh in range(2):
                    pp = ppool.tile([F, 512], F32, name="pp")
                    nc.tensor.matmul(pp[:], wout[:, 0:64],
                                     hstg[:, hh, :, 0:16], start=True,
                                     stop=False)
                    nc.tensor.matmul(pp[:], wout[:, 64:128],
                                     hstg[:, hh, :, 16:32], start=False,
                                     stop=True)
                    ostg = opool.tile([F, 512], F32, name="ostg")
                    nc.vector.tensor_scalar_add(ostg[:], pp[:], bout[:])
                    nc.sync.dma_start(
                        out_d[:, bass.ts(2 * sl + hh, 512)], ostg[:])

    nc.compile()
    return nc


def _to_bf16(x):
    import ml_dtypes
    return np.asarray(x, np.float32).astype(ml_dtypes.bfloat16)


def _perm():
    # gate row order i, f, o, g (PyTorch convention is i, f, g, o)
    return np.concatenate([np.arange(0, 512), np.arange(768, 1024),
                           np.arange(512, 768)])


def host_prep(ts_batch, W_ih_enc, W_hh_enc, b_enc, W_ih_dec, W_hh_dec, b_dec,
              W_out, b_out):
    import ml_dtypes
    perm = _perm()

    def prep_dir(W_ih, W_hh, b):
        Wihp = np.asarray(W_ih, np.float32)[perm] * WSCALE      # [G, F]
        bp = np.asarray(b, np.float32)[perm] * WSCALE
        wih = _to_bf16(np.concatenate([Wihp.T, bp[None, :]], 0))  # [65, G]
        Whp = np.asarray(W_hh, np.float32)[perm] * WSCALE       # [G, H]
        tiles = []
        for c in range(8):
            for j in range(2):
                tiles.append(Whp[128 * c:128 * c + 128,
                                 128 * j:128 * j + 128].T)
        whh = np.ascontiguousarray(np.concatenate(tiles, 1))    # [128, 2048]
        if WHH_FP8:
            whh = whh.astype(ml_dtypes.float8_e3m4)
        else:
            whh = _to_bf16(whh)
        return np.ascontiguousarray(wih), whh

    wih_e, whh_e = prep_dir(W_ih_enc, W_hh_enc, b_enc)
    wih_d, whh_d = prep_dir(W_ih_dec, W_hh_dec, b_dec)
    Wo = np.asarray(W_out, np.float32)                          # [F, H]
    wout = _to_bf16(np.concatenate([Wo[:, 0:128].T, Wo[:, 128:256].T], 1))
    boutv = np.ascontiguousarray(np.asarray(b_out, np.float32)[:, None])
    identm = _to_bf16(np.eye(128, dtype=np.float32))

    ts = np.asarray(ts_batch, np.float32)
    XCOLS = (T + U) * B
    in_maps = []
    for d in range(N_CORES):
        tsl = ts[d * B:(d + 1) * B]                             # [16, T, F]
        xe = np.zeros((F + 1, XCOLS), np.float32)
        xe[:F, :T * B] = tsl.transpose(2, 1, 0).reshape(F, T * B)
        xe[F, :] = 1.0
        xd = np.zeros((F + 1, XCOLS), np.float32)
        xd[:F, :T * B] = tsl[:, ::-1, :].transpose(2, 1, 0).reshape(F, T * B)
        xd[F, :] = 1.0
        in_maps.append({
            "xte": _to_bf16(xe), "xtd": _to_bf16(xd),
            "wih_e": wih_e, "whh_e": whh_e,
            "wih_d": wih_d, "whh_d": whh_d,
            "wout": wout, "bout": boutv, "ident": identm,
        })
    return in_maps


def kernel(ts_batch, W_ih_enc, W_hh_enc, b_enc, W_ih_dec, W_hh_dec, b_dec,
           W_out, b_out):
    from concourse.bass_utils import run_bass_kernel_spmd

    if "nc" not in _CACHE:
        _CACHE["nc"] = _build()
    nc = _CACHE["nc"]

    in_maps = host_prep(ts_batch, W_ih_enc, W_hh_enc, b_enc, W_ih_dec,
                        W_hh_dec, b_dec, W_out, b_out)
    res = run_bass_kernel_spmd(nc, in_maps, core_ids=list(range(N_CORES)))
    outs = []
    for r in res.results:
        o = r["out"].reshape(F, T, B).transpose(2, 1, 0)[:, ::-1, :]
        outs.append(o)
    return np.ascontiguousarray(np.concatenate(outs, 0))


if __name__ == "__main__":
    rng = np.random.default_rng(0)
    demo = {
        "ts_batch": rng.standard_normal((128, T, F), dtype=np.float32),
        "W_ih_enc": rng.standard_normal((G, F), dtype=np.float32) * 0.06,
        "W_hh_enc": rng.standard_normal((G, H), dtype=np.float32) * 0.06,
        "b_enc": rng.standard_normal(G).astype(np.float32) * 0.06,
        "W_ih_dec": rng.standard_normal((G, F), dtype=np.float32) * 0.06,
        "W_hh_dec": rng.standard_normal((G, H), dtype=np.float32) * 0.06,
        "b_dec": rng.standard_normal(G).astype(np.float32) * 0.06,
        "W_out": rng.standard_normal((F, H), dtype=np.float32) * 0.06,
        "b_out": rng.standard_normal(F).astype(np.float32) * 0.06,
    }
    out = kernel(**demo)
    print("kernel output", out.shape, out.dtype, float(np.abs(out).max()))
